# revision 30
# baseline (speedup 1.0000x reference)
"""Trainium2 Bass kernel for nn_CopulaDecoder.

Data-parallel over batch: core b computes batch element b end-to-end.
All activations live transposed (features on partitions, tokens on free dim).
The neighbor-gather softmax is reformulated as a dense count-matrix softmax:
  softmax over the 64 gathered scores == (C * exp(scale*S)) normalized, where
  C[p,v] = sum_n 1[neighbor_index[p,n]==v] * exp(-scale*attn_mask[p,n]).
Scores are small (|scale*S| < ~4 for this model family), so no max-shift.

Optimizations over the original baseline (819us -> 644us):
- KV-MLP mm1/mm2 in fp8-e4m3 with DoubleRow perf mode (K=256 per matmul,
  ~1.4x measured PE throughput).  Final rel err ~4.5e-3 (gate 2e-2).
- The rank-1 true_u row of mm1 emitted as 4 row-banded rank-1 matmuls
  ahead of 4 weight-sharing DR matmuls per psum pair.
- kb3 dropped entirely (per-pred factor cancels in the softmax ratio;
  exact); vb3 folded into the attention output as a per-partition add
  (exact); all other biases folded into activation-evacuation bias slots.
- ds/ff/decoder matmuls single-bf16 (error dominated by fp8 KV anyway).
- Broadcast matmuls (LN rstd/mu, softmax reciprocal) in bf16.
- Attention: [128,1024] score psums shared with the KV psum pool, one
  exp + one C-mult per head pair (count matrix duplicated in SBUF),
  host-precomputed loss one-hot, no final-softmax max shift.
- Layer-1 v-chains emitted after layer-0 attention so the PE backfills
  the scalar(exp)-bound attention windows; 4MB count-matrix DMA issued
  after the l0 chains to keep startup clean.
- Post-l1-attention tail (LN/FF/LN/decoder/logits) split into two
  256-column halves so the halves pipeline across engines.
"""
import os

import numpy as np
import ml_dtypes

B, S, T = 8, 32, 64
V = S * T
P = 512
N = 2 * S
I = 256
H, AD = 8, 32
D = H * AD
M = 256
L = 2
R = 128
SCALE = float(AD) ** -0.5

BF = ml_dtypes.bfloat16
F8 = ml_dtypes.float8_e4m3fn

_BUILT = {}


# ---------------------------------------------------------------------------
# walrus wait-slot workaround (inlined; see dev notes): Tile attaches >1
# semaphore wait to one instruction; many ISA encodings have a single wait
# slot.  Peel excess waits onto injected same-engine InstNoOps.
# ---------------------------------------------------------------------------
def _install_waitfix():
    import bass_rust
    import concourse.mybir as mybir
    import concourse.tile as tile_mod

    if getattr(tile_mod.TileContext, "_waitfix_installed", False):
        return
    limits = {"InstDrain": 1000, "InstEventSemaphore": 1000, "InstCall": 1000}
    counter = [0]
    orig_add = tile_mod.TileContext._add_instruction

    def patched_add(self, inst):
        si = inst.sync_info
        if si is not None:
            limit = limits.get(type(inst).__name__, 1)
            waits = list(si.on_wait)
            if len(waits) > limit:
                keep = waits[-limit:]
                excess = waits[:-limit]
                while excess:
                    chunk, excess = excess[:1], excess[1:]
                    counter[0] += 1
                    nop = bass_rust.InstNoOp(
                        name=f"waitsplit-{counter[0]}", ins=[], outs=[])
                    nop.engine = inst.engine
                    nop.sync_info = mybir.SyncInfo(on_wait=chunk, on_update=[])
                    orig_add(self, nop)
                inst.sync_info = mybir.SyncInfo(
                    on_wait=keep, on_update=list(si.on_update))
        orig_add(self, inst)

    def patched_drain_and_barrier(self, tick_clock, wait_clock):
        from concourse.tile import ScopedClock

        drain_inst = self.nc.sync.drain()
        wait_clock.add_sem_waits(
            drain_inst.ins, ScopedClock({None: tick_clock.global_clock}))
        si = drain_inst.ins.sync_info
        if si is not None and len(si.on_wait) > 1:
            waits = list(si.on_wait)
            drain_inst.ins.sync_info = mybir.SyncInfo(
                on_wait=waits[:1], on_update=list(si.on_update))
            rest = waits[1:]
            while rest:
                chunk, rest = rest[:1], rest[1:]
                nop = self.nc.sync.nop()
                nop.ins.sync_info = mybir.SyncInfo(on_wait=chunk, on_update=[])
        self.nc.all_engine_barrier()
        assert self.sems is not None
        popped = self.nc._tile_sem_poison_stack.pop()
        assert popped is self._sem_poison
        self.nc.clear_and_free_semaphores(list(self.sems.allocated().values()))
        self.nc.all_engine_barrier()

    try:
        import concourse.tile_utils as tile_utils
        tile_utils.max_sbuf_usage = 204 * 1024
    except Exception:
        pass
    tile_mod.TileContext._add_instruction = patched_add
    tile_mod.TileContext._drain_and_barrier = patched_drain_and_barrier
    tile_mod.TileContext._waitfix_installed = True


def _build():
    """Emit the single-core Bass program (SPMD across 8 cores)."""
    import concourse.bass as bass
    import concourse.mybir as mybir
    import concourse.tile as tile

    _install_waitfix()

    F32 = mybir.dt.float32
    BF16 = mybir.dt.bfloat16
    FP8 = mybir.dt.float8e4
    AF = mybir.ActivationFunctionType
    ALU = mybir.AluOpType
    DR = mybir.MatmulPerfMode.DoubleRow

    nc = bass.Bass()

    def din(name, shape, dt=BF16):
        return nc.dram_tensor(name, list(shape), dt, kind="ExternalInput")

    # --- DRAM inputs -------------------------------------------------------
    xt8d = din("xt8d", [128, 2, V], FP8)  # merged.T rows 0:256, DR layout
    u4d = din("u4d", [4, 512])              # row 256 (true_u), 4 copies (bf16)
    ctm = din("ctm", [V, P])              # count matrix transposed  (bf16)
    curh = din("curh", [I, P])            # cur.T (bf16)
    ohtd = din("ohtd", [P, R])            # target one-hot           (bf16)

    kvw = {}
    for pre in ("k", "v"):
        kvw[pre + "18"] = din(pre + "w18", [L, H, 2, 128, M], FP8)
        kvw[pre + "28"] = din(pre + "w28", [L, H, 2, 128, M], FP8)
        kvw[pre + "3"] = din(pre + "w3", [L, H, M, AD])
        kvw[pre + "c4"] = din(pre + "c4", [L, H, 4, M])  # u-row wt, 4 copies
    kb1d = din("kb1d", [L, H, 2, 128], mybir.dt.float32)
    kb2d = din("kb2d", [L, H, 2, 128], mybir.dt.float32)
    vb1d = din("vb1d", [L, H, 2, 128], mybir.dt.float32)
    vb2d = din("vb2d", [L, H, 2, 128], mybir.dt.float32)
    vb3c = din("vb3c", [L, 2, 128, 1], F32)  # vb3 stacked per quad [d-part]

    dswh = din("dswh", [I, D])
    dsbd = din("dsbd", [2, 128, 1], F32)
    ffw1h = din("ffw1h", [L, D, D])
    ffw2h = din("ffw2h", [L, D, D])
    ffb1c = din("ffb1c", [L, 2, 128, 1], F32)
    ffb2c = din("ffb2c", [L, 2, 128, 1], F32)
    ln1gd = din("ln1gd", [L, 2, 128, 1], F32)
    ln1bd = din("ln1bd", [L, 2, 128, 1], F32)
    ln2gd = din("ln2gd", [L, 2, 128, 1], F32)
    ln2bd = din("ln2bd", [L, 2, 128, 1], F32)
    dew1h = din("dew1h", [D, M])
    dew2h = din("dew2h", [M, M])
    dew3h = din("dew3h", [M, R])
    deb1c = din("deb1c", [2, 128, 1], F32)
    deb2c = din("deb2c", [2, 128, 1], F32)
    deb3h = din("deb3h", [1, R])

    oh8d = din("oh8d", [8, D])            # onehot head->rows (bf16)

    out_d = nc.dram_tensor("out", [1, 1], F32, kind="ExternalOutput")

    with tile.TileContext(nc) as tc:
        with (
            tc.tile_pool(name="const", bufs=1) as cpool,
            tc.tile_pool(name="resident", bufs=1) as rpool,
            tc.tile_pool(name="wts", bufs=2) as wpool,
            tc.tile_pool(name="work", bufs=1) as kpool,
            tc.tile_pool(name="psum", bufs=1, space="PSUM") as pp,
        ):
            # --- constants / resident tensors ---------------------------
            ones_r128 = cpool.tile([1, 128], BF16, name="ones_r128")
            nc.vector.memset(ones_r128[:], 1.0)
            ones_r512 = cpool.tile([1, 512], BF16, name="ones_r512")
            nc.vector.memset(ones_r512[:], 1.0)
            ones_c128b = cpool.tile([128, 1], BF16, name="ones_c128b")
            nc.vector.memset(ones_c128b[:], 1.0)
            ones_c128f = cpool.tile([128, 1], F32, name="ones_c128f")
            nc.vector.memset(ones_c128f[:], 1.0)
            eps_t = cpool.tile([1, 1], F32, name="eps_t")
            nc.vector.memset(eps_t[:], 1e-5)
            nlogr_t = cpool.tile([1, 1], F32, name="nlogr_t")
            nc.vector.memset(nlogr_t[:], -float(P) * float(np.log(R)))
            oh8 = cpool.tile([8, D], BF16, name="oh8")
            nc.sync.dma_start(oh8[:], oh8d[:])

            xt8 = rpool.tile([128, 2, V], FP8, name="xt8")
            nc.sync.dma_start(xt8[:], xt8d[:])
            u4 = rpool.tile([128, 512], BF16, name="u4")
            for k4 in range(4):
                nc.sync.dma_start(u4[32 * k4:32 * k4 + 1, :],
                                  u4d[k4:k4 + 1, :])

            # count matrix, duplicated along an inner axis so one [128,1024]
            # multiply covers both heads of a score-pair tile.  DMA is issued
            # later (after the l0 chains) to keep the 4MB transfer off the
            # startup critical path.
            ct2 = rpool.tile([128, 16, 2, P], BF16, name="ct2")

            cur_h = [kpool.tile([128, P], BF16, tag=f"cur_h{q}", bufs=1,
                                name=f"cur_h{q}") for q in range(2)]
            for q in range(2):
                nc.sync.dma_start(cur_h[q][:], curh[128 * q:128 * (q + 1), :])

            # keys (transposed, per (l, quad)) and vals (+ones, per (l,h))
            kt = [[rpool.tile([128, V], BF16, name=f"kt{l}{q}")
                   for q in range(2)] for l in range(L)]
            vals = [[rpool.tile([128, 16, AD + 1], BF16, name=f"vals{l}{h}")
                     for h in range(H)] for l in range(L)]

            # =============================================================
            # Phase helpers
            # =============================================================
            def kv_chain(l, h, pre):
                """One (layer, head, k-or-v) MLP chain over all V rows."""
                w18d, w28d = kvw[pre + "18"], kvw[pre + "28"]
                w3d, wc4d = kvw[pre + "3"], kvw[pre + "c4"]
                cn = f"{pre}{l}{h}"
                w18 = wpool.tile([128, 2, M], FP8, tag="w18", name=f"w18{cn}")
                w28 = wpool.tile([128, 2, M], FP8, tag="w28", name=f"w28{cn}")
                w3a = wpool.tile([128, AD], BF16, tag="w3a", name=f"w3a{cn}")
                w3b = wpool.tile([128, AD], BF16, tag="w3b", name=f"w3b{cn}")
                wc4 = wpool.tile([128, M], BF16, tag="wc4", name=f"wc4{cn}")
                nc.sync.dma_start(
                    w18[:], w18d[l, h].rearrange("i p m -> p i m"))
                nc.sync.dma_start(
                    w28[:], w28d[l, h].rearrange("i p m -> p i m"))
                nc.sync.dma_start(w3a[:], w3d[l, h, 0:128, :])
                nc.sync.dma_start(w3b[:], w3d[l, h, 128:256, :])
                for k4 in range(4):
                    nc.sync.dma_start(wc4[32 * k4:32 * k4 + 1, :],
                                      wc4d[l, h, k4:k4 + 1, :])
                b1d = kb1d if pre == "k" else vb1d
                b2d = kb2d if pre == "k" else vb2d
                b1 = [wpool.tile([128, 1], mybir.dt.float32, tag=f"b1_{fc}",
                                 name=f"b1{cn}_{fc}") for fc in range(2)]
                b2 = [wpool.tile([128, 1], mybir.dt.float32, tag=f"b2_{fc}",
                                 name=f"b2{cn}_{fc}") for fc in range(2)]
                for fc in range(2):
                    nc.sync.dma_start(
                        b1[fc][:], b1d[l, h, fc, :].rearrange("(p o) -> p o", o=1))
                    nc.sync.dma_start(
                        b2[fc][:], b2d[l, h, fc, :].rearrange("(p o) -> p o", o=1))

                # mm1 -> h1 (relu, fp8 DR layout), per (fc, ntp)
                h18 = [kpool.tile([128, 2, 1024], FP8, tag=f"h18_{ntp}",
                                  name=f"h18{cn}{ntp}", bufs=3)
                       for ntp in range(2)]
                for fc in range(2):
                    ps1s = [pp.tile([128, 1024], mybir.dt.float32,
                                    tag="kv", name=f"ps1{cn}{fc}{ntp}",
                                    bufs=2) for ntp in range(2)]
                    # 4 rank-1 true_u rows on 4 distinct 32-row PE bands
                    # (concurrent), then 4 DR matmuls sharing one weight load
                    for ntp in range(2):
                        for j in range(2):
                            nt = ntp * 2 + j
                            nc.tensor.matmul(
                                ps1s[ntp][:, 512 * j:512 * (j + 1)],
                                wc4[32 * nt:32 * nt + 1,
                                    128 * fc:128 * (fc + 1)],
                                u4[32 * nt:32 * nt + 1, :],
                                start=True, stop=False,
                                tile_position=(32 * nt, 0))
                    for ntp in range(2):
                        for j in range(2):
                            nt = ntp * 2 + j
                            nc.tensor.matmul(
                                ps1s[ntp][:, 512 * j:512 * (j + 1)],
                                w18[:, :, 128 * fc:128 * (fc + 1)],
                                xt8[:, :, 512 * nt:512 * (nt + 1)],
                                start=False, stop=True, perf_mode=DR)
                    for ntp in range(2):
                        nc.scalar.activation(h18[ntp][:, fc, :], ps1s[ntp][:],
                                             AF.Relu, bias=b1[fc][:])

                # mm2 -> h2 (relu, bf16)
                h2t = {}
                for fc in range(2):
                    for ntp in range(2):
                        ps2 = pp.tile([128, 1024], mybir.dt.float32,
                                      tag="kv", name=f"ps2{cn}{fc}{ntp}",
                                      bufs=2)
                        for j in range(2):
                            nc.tensor.matmul(
                                ps2[:, 512 * j:512 * (j + 1)],
                                w28[:, :, 128 * fc:128 * (fc + 1)],
                                h18[ntp][:, :, 512 * j:512 * (j + 1)],
                                start=True, stop=True, perf_mode=DR)
                        t = kpool.tile([128, 1024], BF16, tag="h2",
                                       name=f"h2{cn}{fc}{ntp}", bufs=5)
                        if pre == "v" and (fc + ntp) % 2 == 1:
                            nc.scalar.activation(t[:], ps2[:], AF.Relu,
                                                 bias=b2[fc][:])
                        else:
                            nc.vector.tensor_scalar(
                                t[:], ps2[:], b2[fc][:], 0.0, ALU.add, ALU.max)
                        h2t[(fc, ntp)] = t

                if pre == "k":
                    q, hp = h // 4, h % 4
                    for nt in range(4):
                        ntp, j = nt // 2, nt % 2
                        col = slice(512 * j, 512 * (j + 1))
                        psk = pp.tile([AD, 512], mybir.dt.float32,
                                      tag="p512", name=f"psk{cn}{nt}", bufs=2)
                        nc.tensor.matmul(psk[:], w3a[:], h2t[(0, ntp)][:, col],
                                         start=True, stop=False)
                        nc.tensor.matmul(psk[:], w3b[:], h2t[(1, ntp)][:, col],
                                         start=False, stop=True)
                        if l == 0:
                            nc.scalar.copy(
                                kt[l][q][32 * hp:32 * (hp + 1),
                                         512 * nt:512 * (nt + 1)], psk[:])
                        else:
                            nc.vector.tensor_copy(
                                kt[l][q][32 * hp:32 * (hp + 1),
                                         512 * nt:512 * (nt + 1)], psk[:])
                else:
                    psv = pp.tile([128, 512], mybir.dt.float32,
                                  tag="p512", name=f"psv{cn}", bufs=2)
                    for nt in range(4):
                        ntp, j = nt // 2, nt % 2
                        for sv in range(4):
                            svg = nt * 4 + sv
                            vsl = slice(512 * j + 128 * sv, 512 * j + 128 * (sv + 1))
                            osl = slice(32 * svg, 32 * (svg + 1))
                            first = (svg == 0)
                            last = (svg == 15)
                            nc.tensor.matmul(
                                psv[:, osl], h2t[(0, ntp)][:, vsl], w3a[:],
                                start=first, stop=False)
                            nc.tensor.matmul(
                                psv[:, osl], h2t[(1, ntp)][:, vsl], w3b[:],
                                start=False, stop=last)
                    vt = vals[l][h]
                    if h % 2 == 1:
                        nc.scalar.copy(
                            vt[:, :, 0:AD],
                            psv[:].rearrange("p (s d) -> p s d", d=AD))
                    else:
                        nc.vector.tensor_copy(
                            vt[:, :, 0:AD],
                            psv[:].rearrange("p (s d) -> p s d", d=AD))
                    nc.vector.memset(vt[:, :, AD:AD + 1], 1.0)

            # attv tiles (f32) + bf16/lo splits, rotated per layer
            def split_bf(src_tiles, tagp, need_lo=True):
                """f32 [128,P] tiles -> (hi bf16, lo bf16) tiles."""
                his, los = [], []
                for q, s in enumerate(src_tiles):
                    hi = kpool.tile([128, P], BF16, tag=f"{tagp}h{q}",
                                    name=f"{tagp}h{q}", bufs=2, uniquify=True)
                    nc.vector.tensor_copy(hi[:], s[:])
                    his.append(hi)
                    if need_lo:
                        lo = kpool.tile([128, P], BF16, tag=f"{tagp}l{q}",
                                        name=f"{tagp}l{q}", bufs=2, uniquify=True)
                        nc.vector.tensor_tensor(lo[:], s[:], hi[:], ALU.subtract)
                        los.append(lo)
                return his, los

            def layer_norm(xq, gd, bd, l, nm, cols=None, tail=False):
                """T-layout LN over 256 features; returns new f32 tiles.
                cols: list of column slices processed as independent
                pipelined chunks (tail latency hiding)."""
                if cols is None:
                    cols = [slice(0, P)]
                xh = [kpool.tile([128, P], BF16, tag=f"lnxh{q}",
                                 name=f"lnxh{nm}{q}", bufs=2, uniquify=True)
                      for q in range(2)]
                sq = [kpool.tile([128, P], BF16, tag=f"lnsq{q}",
                                 name=f"lnsq{nm}{q}", bufs=1) for q in range(2)]
                mu = kpool.tile([1, P], mybir.dt.float32, tag="lnmu", bufs=1,
                                name=f"lnmu{nm}")
                m2 = kpool.tile([1, P], mybir.dt.float32, tag="lnm2", bufs=1,
                                name=f"lnm2{nm}")
                var = kpool.tile([1, P], mybir.dt.float32, tag="lnvar", bufs=1,
                                 name=f"lnvar{nm}")
                sd = kpool.tile([1, P], mybir.dt.float32, tag="lnsd", bufs=1,
                                name=f"lnsd{nm}")
                rstd = kpool.tile([1, P], BF16, tag="lnrs", bufs=1,
                                  name=f"lnrs{nm}")
                nmu = kpool.tile([1, P], BF16, tag="lnnm", bufs=1,
                                 name=f"lnnm{nm}")
                t1 = [kpool.tile([128, P], mybir.dt.float32, tag=f"lnt{q}",
                                 name=f"lnt{nm}{q}", bufs=1) for q in range(2)]
                outq = [kpool.tile([128, P], mybir.dt.float32, tag=f"attv{q}",
                                   name=f"ln_out{nm}{q}", bufs=2)
                        for q in range(2)]
                gt, bt = [], []
                for q in range(2):
                    g = wpool.tile([128, 1], mybir.dt.float32, tag=f"lng{q}",
                                   name=f"lng{nm}{q}")
                    nc.sync.dma_start(g[:], gd[l, q, :, :])
                    bb = wpool.tile([128, 1], mybir.dt.float32, tag=f"lnb{q}",
                                    name=f"lnb{nm}{q}")
                    nc.sync.dma_start(bb[:], bd[l, q, :, :])
                    gt.append(g); bt.append(bb)
                for cs in cols:
                    for q in range(2):
                        if tail:
                            nc.scalar.copy(xh[q][:, cs], xq[q][:, cs])
                        else:
                            nc.vector.tensor_copy(xh[q][:, cs], xq[q][:, cs])
                    pst = pp.tile([1, 512], mybir.dt.float32, tag="p512",
                                  name=f"lnsum{nm}{cs.start}", bufs=2)
                    w = cs.stop - cs.start
                    nc.tensor.matmul(pst[:, 0:w], ones_c128b[:], xh[0][:, cs],
                                     start=True, stop=False)
                    nc.tensor.matmul(pst[:, 0:w], ones_c128b[:], xh[1][:, cs],
                                     start=False, stop=True)
                    for q in range(2):
                        if tail:
                            nc.scalar.square(sq[q][:, cs], xh[q][:, cs])
                        else:
                            nc.vector.tensor_tensor(sq[q][:, cs], xh[q][:, cs],
                                                    xh[q][:, cs], ALU.mult)
                    psq = pp.tile([1, 512], mybir.dt.float32, tag="p512",
                                  name=f"lnsq{nm}{cs.start}", bufs=2)
                    nc.tensor.matmul(psq[:, 0:w], ones_c128b[:], sq[0][:, cs],
                                     start=True, stop=False)
                    nc.tensor.matmul(psq[:, 0:w], ones_c128b[:], sq[1][:, cs],
                                     start=False, stop=True)
                    nc.scalar.mul(mu[:, cs], pst[:, 0:w], 1.0 / D)
                    nc.vector.tensor_tensor(m2[:, cs], mu[:, cs], mu[:, cs],
                                            ALU.mult)
                    nc.vector.scalar_tensor_tensor(
                        var[:, cs], psq[:, 0:w], 1.0 / D, m2[:, cs],
                        ALU.mult, ALU.subtract)
                    nc.scalar.activation(sd[:, cs], var[:, cs], AF.Sqrt,
                                         bias=eps_t[:])
                    with nc.allow_low_precision(reason="bf16 LN bcast"):
                        nc.vector.reciprocal(rstd[:, cs], sd[:, cs])
                    nc.vector.scalar_tensor_tensor(
                        nmu[:, cs], mu[:, cs], -1.0, rstd[:, cs],
                        ALU.mult, ALU.mult)
                    psa = pp.tile([128, 512], mybir.dt.float32, tag="p512",
                                  name=f"lnA{nm}{cs.start}", bufs=2)
                    nc.tensor.matmul(psa[:, 0:w], ones_r128[:], rstd[:, cs],
                                     start=True, stop=True)
                    psb = pp.tile([128, 512], mybir.dt.float32, tag="p512",
                                  name=f"lnB{nm}{cs.start}", bufs=2)
                    nc.tensor.matmul(psb[:, 0:w], ones_r128[:], nmu[:, cs],
                                     start=True, stop=True)
                    for q in range(2):
                        nc.vector.tensor_tensor(t1[q][:, cs], xq[q][:, cs],
                                                psa[:, 0:w], ALU.mult)
                        nc.vector.tensor_tensor(t1[q][:, cs], t1[q][:, cs],
                                                psb[:, 0:w], ALU.add)
                        if tail:
                            nc.scalar.activation(outq[q][:, cs], t1[q][:, cs],
                                                 AF.Identity, scale=gt[q][:],
                                                 bias=bt[q][:])
                        else:
                            nc.vector.tensor_scalar(outq[q][:, cs],
                                                    t1[q][:, cs],
                                                    gt[q][:], bt[q][:],
                                                    ALU.mult, ALU.add)
                return outq

            def attention(l, attv):
                """One attention block (scores/softmax/values) -> xres."""
                qt_h, _ = split_bf(attv, "qt", need_lo=False)
                numer = [kpool.tile([128, P], mybir.dt.float32, tag=f"num{q}",
                                    name=f"numer{l}{q}", bufs=1)
                         for q in range(2)]
                dn8 = kpool.tile([8, P], mybir.dt.float32, tag="dn8",
                                 name=f"dn8{l}", bufs=1)
                for pair in range(4):
                    h0 = 2 * pair
                    h1 = h0 + 1
                    q = h0 // 4
                    b0, b1r = 32 * (h0 % 4), 32 * (h1 % 4)
                    psA = pp.tile([128, 1024], mybir.dt.float32, tag="acc",
                                  name=f"psA{l}{pair}", bufs=1)
                    for vc in range(16):
                        pss = pp.tile([128, 1024], mybir.dt.float32,
                                      tag="kv", name=f"pss{l}{pair}{vc}",
                                      bufs=2)
                        for hi, (hh, bb) in enumerate(((h0, b0), (h1, b1r))):
                            tp = (bb, 0) if bb == 96 else None
                            nc.tensor.matmul(
                                pss[:, 512 * hi:512 * (hi + 1)],
                                kt[l][q][bb:bb + 32, 128 * vc:128 * (vc + 1)],
                                qt_h[q][bb:bb + 32, :],
                                start=True, stop=True, tile_position=tp)
                        e = kpool.tile([128, 1024], BF16, tag="ebuf",
                                       name=f"e{l}{pair}{vc}", bufs=2)
                        nc.scalar.activation(e[:], pss[:], AF.Exp,
                                             scale=SCALE)
                        ce = kpool.tile([128, 1024], BF16, tag="cebuf",
                                        name=f"ce{l}{pair}{vc}", bufs=2)
                        nc.vector.tensor_tensor(
                            ce[:], e[:],
                            ct2[:, vc, :, :].rearrange("p a q -> p (a q)"),
                            ALU.mult)
                        nc.tensor.matmul(
                            psA[0:AD + 1, 0:512],
                            vals[l][h0][:, vc, :], ce[:, 0:512],
                            start=(vc == 0), stop=(vc == 15))
                        nc.tensor.matmul(
                            psA[64:64 + AD + 1, 512:1024],
                            vals[l][h1][:, vc, :], ce[:, 512:1024],
                            start=(vc == 0), stop=(vc == 15),
                            tile_position=(0, 64))
                    nc.vector.tensor_copy(numer[q][b0:b0 + 32, :],
                                          psA[0:32, 0:512])
                    nc.vector.tensor_copy(numer[q][b1r:b1r + 32, :],
                                          psA[64:96, 512:1024])
                    for hh, prow, csl in ((h0, 32, slice(0, 512)),
                                          (h1, 96, slice(512, 1024))):
                        dtmp = kpool.tile([1, P], mybir.dt.float32,
                                          tag="dntmp", bufs=2, name=f"dtmp{l}{hh}",
                                          uniquify=True)
                        nc.vector.tensor_copy(dtmp[:], psA[prow:prow + 1, csl])
                        nc.sync.dma_start(dn8[hh:hh + 1, :], dtmp[:])

                rd8 = kpool.tile([8, P], BF16, tag="rd8",
                                 name=f"rd8{l}", bufs=1)
                with nc.allow_low_precision(reason="bf16 softmax recip bcast"):
                    nc.vector.reciprocal(rd8[:], dn8[:])
                vb3q = [wpool.tile([128, 1], mybir.dt.float32, tag=f"vb3{q}",
                                   name=f"vb3{l}{q}") for q in range(2)]
                for q in range(2):
                    nc.sync.dma_start(vb3q[q][:], vb3c[l, q, :, :])
                xres = []
                for q in range(2):
                    psrb = pp.tile([128, 512], mybir.dt.float32, tag="p512",
                                   name=f"psrb{l}{q}", bufs=2)
                    nc.tensor.matmul(psrb[:], oh8[:, 128 * q:128 * (q + 1)],
                                     rd8[:], start=True, stop=True)
                    t1 = kpool.tile([128, P], mybir.dt.float32, tag=f"xres{q}",
                                    name=f"xres{l}{q}", bufs=1)
                    nc.vector.tensor_tensor(t1[:], numer[q][:], psrb[:], ALU.mult)
                    nc.vector.tensor_scalar(t1[:], t1[:], vb3q[q][:], None,
                                            ALU.add)
                    nc.vector.tensor_tensor(t1[:], attv[q][:], t1[:], ALU.add)
                    xres.append(t1)
                return xres

            def ff_block(l, attv, cols=None, tail=False):
                if cols is None:
                    cols = [slice(0, P)]
                fw1h = [wpool.tile([128, D], BF16, tag=f"fw1h{kc}",
                                   name=f"fw1h{l}{kc}") for kc in range(2)]
                fw2h = [wpool.tile([128, D], BF16, tag=f"fw2h{kc}",
                                   name=f"fw2h{l}{kc}") for kc in range(2)]
                for kc in range(2):
                    sl = slice(128 * kc, 128 * (kc + 1))
                    nc.sync.dma_start(fw1h[kc][:], ffw1h[l, sl, :])
                    nc.sync.dma_start(fw2h[kc][:], ffw2h[l, sl, :])
                fb1 = [wpool.tile([128, 1], F32, tag=f"fb1_{fc}",
                                  name=f"fb1{l}{fc}") for fc in range(2)]
                fb2 = [wpool.tile([128, 1], F32, tag=f"fb2_{fc}",
                                  name=f"fb2{l}{fc}") for fc in range(2)]
                for fc in range(2):
                    nc.sync.dma_start(fb1[fc][:], ffb1c[l, fc, :, :])
                    nc.sync.dma_start(fb2[fc][:], ffb2c[l, fc, :, :])

                av_h = [kpool.tile([128, P], BF16, tag=f"ffsh{q}",
                                   name=f"ffsh{l}{q}", bufs=2, uniquify=True)
                        for q in range(2)]
                hh_t = [kpool.tile([128, P], BF16, tag=f"ffhh{fc}",
                                   name=f"ffhh{l}{fc}", bufs=2)
                        for fc in range(2)]
                xres2 = [kpool.tile([128, P], mybir.dt.float32, tag=f"xres{fc}",
                                    name=f"xr2{l}{fc}", bufs=1)
                         for fc in range(2)]
                t2a = [kpool.tile([128, P], mybir.dt.float32, tag=f"t2a{fc}",
                                  name=f"t2a{l}{fc}", bufs=1)
                       for fc in range(2)]
                for cs in cols:
                    w = cs.stop - cs.start
                    for q in range(2):
                        if tail:
                            nc.scalar.copy(av_h[q][:, cs], attv[q][:, cs])
                        else:
                            nc.vector.tensor_copy(av_h[q][:, cs], attv[q][:, cs])
                    for fc in range(2):
                        psf = pp.tile([128, 512], mybir.dt.float32, tag="p512",
                                      name=f"psff1{l}{fc}{cs.start}", bufs=2)
                        for kc in range(2):
                            nc.tensor.matmul(
                                psf[:, 0:w], fw1h[kc][:, 128 * fc:128 * (fc + 1)],
                                av_h[kc][:, cs], start=(kc == 0), stop=(kc == 1))
                        nc.scalar.activation(hh_t[fc][:, cs], psf[:, 0:w],
                                             AF.Relu, bias=fb1[fc][:])
                    for fc in range(2):
                        psf2 = pp.tile([128, 512], mybir.dt.float32, tag="p512",
                                       name=f"psff2{l}{fc}{cs.start}", bufs=2)
                        for kc in range(2):
                            nc.tensor.matmul(
                                psf2[:, 0:w], fw2h[kc][:, 128 * fc:128 * (fc + 1)],
                                hh_t[kc][:, cs], start=(kc == 0), stop=(kc == 1))
                        nc.scalar.activation(t2a[fc][:, cs], psf2[:, 0:w],
                                             AF.Identity, bias=fb2[fc][:])
                        nc.vector.tensor_tensor(xres2[fc][:, cs],
                                                attv[fc][:, cs],
                                                t2a[fc][:, cs], ALU.add)
                return xres2

            # =============================================================
            # Emit program
            # =============================================================
            # ds projection: attv0 = cur @ ds_W + ds_b   (T-layout out)
            dsw_h = [cpool.tile([128, D], BF16, name=f"dswh{kc}") for kc in range(2)]
            for kc in range(2):
                nc.sync.dma_start(dsw_h[kc][:], dswh[128 * kc:128 * (kc + 1), :])
            dsb_q = [cpool.tile([128, 1], F32, name=f"dsb{q}") for q in range(2)]
            for q in range(2):
                nc.sync.dma_start(dsb_q[q][:], dsbd[q, :, :])

            attv = []
            for q in range(2):
                psd = pp.tile([128, 512], mybir.dt.float32, tag="p512",
                              name=f"psds{q}", bufs=2)
                for kc in range(2):
                    nc.tensor.matmul(psd[:], dsw_h[kc][:, 128 * q:128 * (q + 1)],
                                     cur_h[kc][:], start=(kc == 0),
                                     stop=(kc == 1))
                o = kpool.tile([128, P], mybir.dt.float32, tag=f"attv{q}",
                               name=f"attv0{q}", bufs=2)
                nc.scalar.activation(o[:], psd[:], AF.Identity,
                                     bias=dsb_q[q][:])
                attv.append(o)

            # l0 KV, then l1 keys
            for q in range(2):
                for hp in range(4):
                    kv_chain(0, 4 * q + hp, "k")
                for hp in range(4):
                    kv_chain(0, 4 * q + hp, "v")
            for cc in range(2):
                nc.sync.dma_start(
                    ct2[:, :, cc, :], ctm.rearrange("(c p) q -> p c q", p=128))
            for q in range(2):
                for hp in range(4):
                    kv_chain(1, 4 * q + hp, "k")

            # layer 0 attention + FF (overlaps l1 v-chains below)
            xres = attention(0, attv)
            attv = layer_norm(xres, ln1gd, ln1bd, 0, "ln1_0")

            # l1 v-chains: emitted after l0 attention so the scheduler
            # backfills PE bubbles during the scalar-bound attention window;
            # the q1 half goes after LN2 to fill the pre-l1-attention dip
            for hp in range(4):
                kv_chain(1, hp, "v")

            xres2 = ff_block(0, attv, tail=True)
            attv = layer_norm(xres2, ln2gd, ln2bd, 0, "ln2_0", tail=True)
            for hp in range(4):
                kv_chain(1, 4 + hp, "v")

            # layer 1
            HALVES = [slice(0, 256), slice(256, 512)]
            xres = attention(1, attv)
            attv = layer_norm(xres, ln1gd, ln1bd, 1, "ln1_1", cols=HALVES,
                              tail=True)
            xres2 = ff_block(1, attv, cols=HALVES, tail=True)
            attv = layer_norm(xres2, ln2gd, ln2bd, 1, "ln2_1", cols=HALVES,
                              tail=True)

            # ---- decoder --------------------------------------------------
            dw1h = [cpool.tile([128, M], BF16, name=f"dw1h{kc}") for kc in range(2)]
            dw2h = [cpool.tile([128, M], BF16, name=f"dw2h{kc}") for kc in range(2)]
            dw3h = [cpool.tile([128, R], BF16, name=f"dw3h{kc}") for kc in range(2)]
            for kc in range(2):
                sl = slice(128 * kc, 128 * (kc + 1))
                nc.sync.dma_start(dw1h[kc][:], dew1h[sl, :])
                nc.sync.dma_start(dw2h[kc][:], dew2h[sl, :])
                nc.sync.dma_start(dw3h[kc][:], dew3h[sl, :])
            db1 = [cpool.tile([128, 1], F32, name=f"db1{fc}") for fc in range(2)]
            db2 = [cpool.tile([128, 1], F32, name=f"db2{fc}") for fc in range(2)]
            for fc in range(2):
                nc.sync.dma_start(db1[fc][:], deb1c[fc, :, :])
                nc.sync.dma_start(db2[fc][:], deb2c[fc, :, :])
            db3h = cpool.tile([1, R], BF16, name="db3h")
            nc.sync.dma_start(db3h[:], deb3h[:])

            de_h = [kpool.tile([128, P], BF16, tag=f"ffsh{q}",
                                name=f"desh{q}", bufs=2, uniquify=True)
                    for q in range(2)]
            d1h = [kpool.tile([128, P], BF16, tag=f"d1h{fc}",
                              name=f"d1h{fc}", bufs=1) for fc in range(2)]
            d2h = [kpool.tile([128, P], BF16, tag=f"d2h{fc}",
                              name=f"d2h{fc}", bufs=1) for fc in range(2)]
            for cs in HALVES:
                w = cs.stop - cs.start
                for q in range(2):
                    nc.scalar.copy(de_h[q][:, cs], attv[q][:, cs])
                for fc in range(2):
                    psd1 = pp.tile([128, 512], mybir.dt.float32, tag="p512",
                                   name=f"psde1{fc}{cs.start}", bufs=2)
                    for kc in range(2):
                        nc.tensor.matmul(
                            psd1[:, 0:w], dw1h[kc][:, 128 * fc:128 * (fc + 1)],
                            de_h[kc][:, cs], start=(kc == 0), stop=(kc == 1))
                    nc.scalar.activation(d1h[fc][:, cs], psd1[:, 0:w],
                                         AF.Relu, bias=db1[fc][:])
                for fc in range(2):
                    psd2 = pp.tile([128, 512], mybir.dt.float32, tag="p512",
                                   name=f"psde2{fc}{cs.start}", bufs=2)
                    for kc in range(2):
                        nc.tensor.matmul(
                            psd2[:, 0:w], dw2h[kc][:, 128 * fc:128 * (fc + 1)],
                            d1h[kc][:, cs], start=(kc == 0), stop=(kc == 1))
                    nc.scalar.activation(d2h[fc][:, cs], psd2[:, 0:w],
                                         AF.Relu, bias=db2[fc][:])

            # logits row-major [p, R] per 128-p chunk + loss
            t4 = kpool.tile([128, 4], mybir.dt.float32, tag="t4",
                            name="t4", bufs=1)
            for pc in range(4):
                psl = pp.tile([128, R], mybir.dt.float32, tag="p512",
                              name=f"pslog{pc}", bufs=2)
                nc.tensor.matmul(psl[:], ones_r128[:], db3h[:],
                                 start=True, stop=False)
                psl_sl = slice(128 * pc, 128 * (pc + 1))
                for kc in range(2):
                    nc.tensor.matmul(psl[:], d2h[kc][:, psl_sl], dw3h[kc][:],
                                     start=False, stop=(kc == 1))
                escr = kpool.tile([128, R], mybir.dt.float32, tag="escr", bufs=1,
                                  name=f"escr{pc}")
                se = kpool.tile([128, 1], mybir.dt.float32, tag="se",
                                name=f"se{pc}", bufs=2)
                nc.scalar.activation(escr[:], psl[:], AF.Exp,
                                     accum_out=se[:])
                ls = kpool.tile([128, 1], mybir.dt.float32, tag="ls",
                                name=f"ls{pc}", bufs=2)
                nc.scalar.activation(ls[:], se[:], AF.Ln)
                oht = kpool.tile([128, R], BF16, tag="ohh", bufs=2,
                                 name=f"oht{pc}")
                nc.sync.dma_start(oht[:], ohtd[psl_sl, :])
                scr2 = kpool.tile([128, R], mybir.dt.float32, tag="scr2", bufs=1,
                                  name=f"scr2{pc}")
                pk = kpool.tile([128, 1], mybir.dt.float32, tag="pk",
                                name=f"pk{pc}", bufs=2)
                nc.vector.scalar_tensor_tensor(
                    scr2[:], psl[:], 1.0, oht[:], ALU.mult, ALU.mult,
                    accum_out=pk[:])
                nc.vector.tensor_tensor(t4[:, pc:pc + 1], pk[:], ls[:],
                                        ALU.subtract)
            pspr = pp.tile([1, 4], mybir.dt.float32, tag="p512", name="pspr", bufs=2)
            nc.tensor.matmul(pspr[:], ones_c128f[:], t4[:], start=True, stop=True)
            pr4 = kpool.tile([1, 4], mybir.dt.float32, tag="pr4",
                             name="pr4", bufs=1)
            nc.vector.tensor_copy(pr4[:], pspr[:])
            s1 = kpool.tile([1, 1], mybir.dt.float32, tag="s1",
                            name="s1", bufs=1)
            nc.vector.tensor_reduce(s1[:], pr4[:], mybir.AxisListType.X, ALU.add)
            outt = kpool.tile([1, 1], mybir.dt.float32, tag="outt",
                              name="outt", bufs=1)
            nc.scalar.activation(outt[:], s1[:], AF.Identity,
                                 bias=nlogr_t[:], scale=-1.0)
            nc.sync.dma_start(out_d[:], outt[:])

    return nc


def _split(x):
    h = np.asarray(x, np.float32).astype(BF)
    lo = (np.asarray(x, np.float32) - h.astype(np.float32)).astype(BF)
    return h, lo


def _maybe_enable_trace():
    """Optional NTFF profiling under axon (KERNEL_TRACE=1); best-effort."""
    try:
        import sys
        import types

        import antenv

        if "antenv.axon_hooks" not in sys.modules:
            mod = types.ModuleType("antenv.axon_hooks")
            mod._hook = None
            mod.set_axon_ntff_profile_hook = lambda h: setattr(mod, "_hook", h)
            mod.get_axon_ntff_profile_hook = lambda: mod._hook
            sys.modules["antenv.axon_hooks"] = mod
            antenv.axon_hooks = mod
            from trn_agent_boot.trn_boot import _ntff_profile_via_ctypes

            mod._hook = _ntff_profile_via_ctypes("/opt/axon/libaxon_pjrt.so")
        import concourse.bass_utils as _bu

        _bu.upload_artifacts = lambda tmpdir: f"file://{tmpdir}"
        return True
    except Exception:
        return False


LAST_RESULT = {}


def kernel(**inputs):
    from concourse.bass_utils import run_bass_kernel_spmd

    if "nc" not in _BUILT:
        _BUILT["nc"] = _build()
    nc = _BUILT["nc"]

    f32 = lambda a: np.ascontiguousarray(np.asarray(a, np.float32))
    bf = lambda a: np.ascontiguousarray(np.asarray(a, np.float32)).astype(BF)
    f8 = lambda a: np.ascontiguousarray(np.asarray(a, np.float32)).astype(F8)

    enc = f32(inputs["encoded"])                      # [B,V,I]
    tu = f32(inputs["true_u"])                        # [B,V,1]
    mask = f32(inputs["attn_mask"])                   # [P,N]
    pp_ = np.asarray(inputs["pred_points"]).astype(np.int64)
    ni = np.asarray(inputs["neighbor_index"]).astype(np.int64)

    # count matrix C[p, v]
    C = np.zeros((P, V), np.float32)
    np.add.at(C, (np.repeat(np.arange(P), N), ni.ravel()),
              np.exp(-SCALE * mask).ravel().astype(np.float32))
    ctm = np.ascontiguousarray(C.T).astype(BF)        # [V, P]

    shared = {"ctm": ctm}
    for pre in ("k", "v"):
        W1 = f32(inputs[pre + "W1"])                  # [L,H,257,M]
        W2 = f32(inputs[pre + "W2"])                  # [L,H,M,M]
        shared[pre + "w18"] = f8(
            W1[:, :, :256, :].reshape(L, H, 2, 128, M))
        shared[pre + "w28"] = f8(W2.reshape(L, H, 2, 128, M))
        shared[pre + "w3"] = bf(inputs[pre + "W3"])
        shared[pre + "c4"] = bf(
            np.broadcast_to(W1[:, :, 256:257, :], (L, H, 4, M)))
    shared["kb1d"] = f32(inputs["kb1"]).reshape(L, H, 2, 128)
    shared["kb2d"] = f32(inputs["kb2"]).reshape(L, H, 2, 128)
    shared["vb1d"] = f32(inputs["vb1"]).reshape(L, H, 2, 128)
    shared["vb2d"] = f32(inputs["vb2"]).reshape(L, H, 2, 128)
    shared["vb3c"] = np.ascontiguousarray(
        f32(inputs["vb3"]).reshape(L, 2, 128, 1))     # [l, q, hp*32+d, 1]

    for nm, key in (("dsw", "ds_W"), ("dew1", "de_W1"), ("dew2", "de_W2"),
                    ("dew3", "de_W3"), ("ffw1", "ff_W1"), ("ffw2", "ff_W2")):
        shared[nm + "h"] = bf(inputs[key])
    shared["dsbd"] = f32(inputs["ds_b"]).reshape(2, 128, 1)
    shared["ffb1c"] = f32(inputs["ff_b1"]).reshape(L, 2, 128, 1)
    shared["ffb2c"] = f32(inputs["ff_b2"]).reshape(L, 2, 128, 1)
    shared["deb1c"] = f32(inputs["de_b1"]).reshape(2, 128, 1)
    shared["deb2c"] = f32(inputs["de_b2"]).reshape(2, 128, 1)
    shared["deb3h"] = bf(inputs["de_b3"]).reshape(1, R)
    shared["ln1gd"] = f32(inputs["ln1_g"]).reshape(L, 2, 128, 1)
    shared["ln1bd"] = f32(inputs["ln1_b"]).reshape(L, 2, 128, 1)
    shared["ln2gd"] = f32(inputs["ln2_g"]).reshape(L, 2, 128, 1)
    shared["ln2bd"] = f32(inputs["ln2_b"]).reshape(L, 2, 128, 1)

    oh8f = np.zeros((8, D), np.float32)
    for hh in range(8):
        base = 128 * (hh // 4) + 32 * (hh % 4)
        oh8f[hh, base:base + 32] = 1.0
    shared["oh8d"] = oh8f.astype(BF)

    in_maps = []
    for b in range(B):
        merged = np.concatenate([enc[b], tu[b]], axis=1)  # [V, 257]
        mt = np.ascontiguousarray(merged.T)               # [257, V]
        cur = enc[b][pp_, :]                              # [P, I]
        curt = np.ascontiguousarray(cur.T)                # [I, P]
        m = dict(shared)
        m["xt8d"] = np.ascontiguousarray(
            mt[0:256].reshape(2, 128, V).transpose(1, 0, 2)).astype(F8)
        m["u4d"] = np.ascontiguousarray(
            mt[256].reshape(4, 512)).astype(BF)
        m["curh"] = curt.astype(BF)
        tgt = np.clip(np.floor(tu[b][pp_, 0] * R).astype(np.int64), 0, R - 1)
        ohp = np.zeros((P, R), np.float32)
        ohp[np.arange(P), tgt] = 1.0
        m["ohtd"] = ohp.astype(BF)
        in_maps.append(m)

    trace = os.environ.get("KERNEL_TRACE") == "1" and _maybe_enable_trace()
    res = run_bass_kernel_spmd(
        nc, in_maps, core_ids=list(range(B)), trace=trace,
        trace_cores=list(range(B)) if trace else None)
    LAST_RESULT["res"] = res
    if trace and res.exec_time_ns is not None:
        print(f"HW exec time: {res.exec_time_ns} ns "
              f"(mean {res.mean_exec_time_ns} ns, "
              f"slowest core {res.max_exec_time_core_id})")
    out = np.array([res.results[b]["out"][0, 0] for b in range(B)], np.float32)
    return out


# revision 31
# speedup vs baseline: 1.2057x; 1.2057x over previous
"""Trainium2 Bass kernel for nn_CopulaDecoder.

Data-parallel over batch: core b computes batch element b end-to-end.
All activations live transposed (features on partitions, tokens on free dim).
The neighbor-gather softmax is reformulated as a dense count-matrix softmax:
  softmax over the 64 gathered scores == (C * exp(scale*S)) normalized, where
  C[p,v] = sum_n 1[neighbor_index[p,n]==v] * exp(-scale*attn_mask[p,n]).
Scores are small (|scale*S| < ~4 for this model family), so no max-shift.

Optimizations over the original baseline (819us -> 644us):
- KV-MLP mm1/mm2 in fp8-e4m3 with DoubleRow perf mode (K=256 per matmul,
  ~1.4x measured PE throughput).  Final rel err ~4.5e-3 (gate 2e-2).
- The rank-1 true_u row of mm1 emitted as 4 row-banded rank-1 matmuls
  ahead of 4 weight-sharing DR matmuls per psum pair.
- kb3 dropped entirely (per-pred factor cancels in the softmax ratio;
  exact); vb3 folded into the attention output as a per-partition add
  (exact); all other biases folded into activation-evacuation bias slots.
- ds/ff/decoder matmuls single-bf16 (error dominated by fp8 KV anyway).
- Broadcast matmuls (LN rstd/mu, softmax reciprocal) in bf16.
- Attention: [128,1024] score psums shared with the KV psum pool, one
  exp + one C-mult per head pair (count matrix duplicated in SBUF),
  host-precomputed loss one-hot, no final-softmax max shift.
- Layer-1 v-chains emitted after layer-0 attention so the PE backfills
  the scalar(exp)-bound attention windows; 4MB count-matrix DMA issued
  after the l0 chains to keep startup clean.
- Post-l1-attention tail (LN/FF/LN/decoder/logits) split into two
  256-column halves so the halves pipeline across engines.
"""
import os

import numpy as np
import ml_dtypes

B, S, T = 8, 32, 64
V = S * T
P = 512
N = 2 * S
I = 256
H, AD = 8, 32
D = H * AD
M = 256
L = 2
R = 128
SCALE = float(AD) ** -0.5

BF = ml_dtypes.bfloat16
F8 = ml_dtypes.float8_e4m3fn

_BUILT = {}


# ---------------------------------------------------------------------------
# walrus wait-slot workaround (inlined; see dev notes): Tile attaches >1
# semaphore wait to one instruction; many ISA encodings have a single wait
# slot.  Peel excess waits onto injected same-engine InstNoOps.
# ---------------------------------------------------------------------------
def _install_waitfix():
    import bass_rust
    import concourse.mybir as mybir
    import concourse.tile as tile_mod

    if getattr(tile_mod.TileContext, "_waitfix_installed", False):
        return
    limits = {"InstDrain": 1000, "InstEventSemaphore": 1000, "InstCall": 1000}
    counter = [0]
    orig_add = tile_mod.TileContext._add_instruction

    def patched_add(self, inst):
        si = inst.sync_info
        if si is not None:
            limit = limits.get(type(inst).__name__, 1)
            waits = list(si.on_wait)
            if len(waits) > limit:
                keep = waits[-limit:]
                excess = waits[:-limit]
                while excess:
                    chunk, excess = excess[:1], excess[1:]
                    counter[0] += 1
                    nop = bass_rust.InstNoOp(
                        name=f"waitsplit-{counter[0]}", ins=[], outs=[])
                    nop.engine = inst.engine
                    nop.sync_info = mybir.SyncInfo(on_wait=chunk, on_update=[])
                    orig_add(self, nop)
                inst.sync_info = mybir.SyncInfo(
                    on_wait=keep, on_update=list(si.on_update))
        orig_add(self, inst)

    def patched_drain_and_barrier(self, tick_clock, wait_clock):
        from concourse.tile import ScopedClock

        drain_inst = self.nc.sync.drain()
        wait_clock.add_sem_waits(
            drain_inst.ins, ScopedClock({None: tick_clock.global_clock}))
        si = drain_inst.ins.sync_info
        if si is not None and len(si.on_wait) > 1:
            waits = list(si.on_wait)
            drain_inst.ins.sync_info = mybir.SyncInfo(
                on_wait=waits[:1], on_update=list(si.on_update))
            rest = waits[1:]
            while rest:
                chunk, rest = rest[:1], rest[1:]
                nop = self.nc.sync.nop()
                nop.ins.sync_info = mybir.SyncInfo(on_wait=chunk, on_update=[])
        self.nc.all_engine_barrier()
        assert self.sems is not None
        popped = self.nc._tile_sem_poison_stack.pop()
        assert popped is self._sem_poison
        self.nc.clear_and_free_semaphores(list(self.sems.allocated().values()))
        self.nc.all_engine_barrier()

    try:
        import concourse.tile_utils as tile_utils
        tile_utils.max_sbuf_usage = 204 * 1024
    except Exception:
        pass
    tile_mod.TileContext._add_instruction = patched_add
    tile_mod.TileContext._drain_and_barrier = patched_drain_and_barrier
    tile_mod.TileContext._waitfix_installed = True


def _build():
    """Emit the single-core Bass program (SPMD across 8 cores)."""
    import concourse.bass as bass
    import concourse.mybir as mybir
    import concourse.tile as tile

    _install_waitfix()

    F32 = mybir.dt.float32
    BF16 = mybir.dt.bfloat16
    FP8 = mybir.dt.float8e4
    AF = mybir.ActivationFunctionType
    ALU = mybir.AluOpType
    DR = mybir.MatmulPerfMode.DoubleRow

    nc = bass.Bass()

    def din(name, shape, dt=BF16):
        return nc.dram_tensor(name, list(shape), dt, kind="ExternalInput")

    # --- DRAM inputs -------------------------------------------------------
    xt8d = din("xt8d", [128, 2, V], FP8)  # merged.T rows 0:256, DR layout
    u4d = din("u4d", [4, 512])              # row 256 (true_u), 4 copies (bf16)
    ctm = din("ctm", [V, P])              # count matrix transposed  (bf16)
    curh = din("curh", [I, P])            # cur.T (bf16)
    ohtd = din("ohtd", [P, R])            # target one-hot           (bf16)

    kvw = {}
    for pre in ("k", "v"):
        kvw[pre + "18"] = din(pre + "w18", [L, H, 2, 128, M], FP8)
        kvw[pre + "28"] = din(pre + "w28", [L, H, 2, 128, M], FP8)
        kvw[pre + "3"] = din(pre + "w3", [L, H, M, AD])
        kvw[pre + "c4"] = din(pre + "c4", [L, H, 4, M])  # u-row wt, 4 copies
    kb1d = din("kb1d", [L, H, 2, 128], mybir.dt.float32)
    kb2d = din("kb2d", [L, H, 2, 128], mybir.dt.float32)
    vb1d = din("vb1d", [L, H, 2, 128], mybir.dt.float32)
    vb2d = din("vb2d", [L, H, 2, 128], mybir.dt.float32)
    vb3c = din("vb3c", [L, 2, 128, 1], F32)  # vb3 stacked per quad [d-part]

    dswh = din("dswh", [I, D])
    dsbd = din("dsbd", [2, 128, 1], F32)
    ffw1h = din("ffw1h", [L, D, D])
    ffw2h = din("ffw2h", [L, D, D])
    ffb1c = din("ffb1c", [L, 2, 128, 1], F32)
    ffb2c = din("ffb2c", [L, 2, 128, 1], F32)
    ln1gd = din("ln1gd", [L, 2, 128, 1], F32)
    ln1bd = din("ln1bd", [L, 2, 128, 1], F32)
    ln2gd = din("ln2gd", [L, 2, 128, 1], F32)
    ln2bd = din("ln2bd", [L, 2, 128, 1], F32)
    dew1h = din("dew1h", [D, M])
    dew2h = din("dew2h", [M, M])
    dew3h = din("dew3h", [M, R])
    deb1c = din("deb1c", [2, 128, 1], F32)
    deb2c = din("deb2c", [2, 128, 1], F32)
    deb3h = din("deb3h", [1, R])

    oh8d = din("oh8d", [8, D])            # onehot head->rows (bf16)

    out_d = nc.dram_tensor("out", [1, 1], F32, kind="ExternalOutput")

    with tile.TileContext(nc) as tc:
        with (
            tc.tile_pool(name="const", bufs=1) as cpool,
            tc.tile_pool(name="resident", bufs=1) as rpool,
            tc.tile_pool(name="wts", bufs=2) as wpool,
            tc.tile_pool(name="work", bufs=1) as kpool,
            tc.tile_pool(name="psum", bufs=1, space="PSUM") as pp,
        ):
            # --- constants / resident tensors ---------------------------
            ones_r128 = cpool.tile([1, 128], BF16, name="ones_r128")
            nc.vector.memset(ones_r128[:], 1.0)
            ones_r512 = cpool.tile([1, 512], BF16, name="ones_r512")
            nc.vector.memset(ones_r512[:], 1.0)
            ones_c128b = cpool.tile([128, 1], BF16, name="ones_c128b")
            nc.vector.memset(ones_c128b[:], 1.0)
            ones_c128f = cpool.tile([128, 1], F32, name="ones_c128f")
            nc.vector.memset(ones_c128f[:], 1.0)
            eps_t = cpool.tile([1, 1], F32, name="eps_t")
            nc.vector.memset(eps_t[:], 1e-5)
            nlogr_t = cpool.tile([1, 1], F32, name="nlogr_t")
            nc.vector.memset(nlogr_t[:], -float(P) * float(np.log(R)))
            oh8 = cpool.tile([8, D], BF16, name="oh8")
            nc.sync.dma_start(oh8[:], oh8d[:])

            xt8 = rpool.tile([128, 2, V], FP8, name="xt8")
            nc.sync.dma_start(xt8[:], xt8d[:])
            u4 = rpool.tile([128, 512], BF16, name="u4")
            for k4 in range(4):
                nc.sync.dma_start(u4[32 * k4:32 * k4 + 1, :],
                                  u4d[k4:k4 + 1, :])

            # count matrix, duplicated along an inner axis so one [128,1024]
            # multiply covers both heads of a score-pair tile.  DMA is issued
            # later (after the l0 chains) to keep the 4MB transfer off the
            # startup critical path.
            ct2 = rpool.tile([128, 16, 2, P], BF16, name="ct2")

            cur_h = [kpool.tile([128, P], BF16, tag=f"cur_h{q}", bufs=1,
                                name=f"cur_h{q}") for q in range(2)]
            for q in range(2):
                nc.sync.dma_start(cur_h[q][:], curh[128 * q:128 * (q + 1), :])

            # keys (transposed, per (l, quad)) and vals (+ones, per (l,h))
            kt = [[rpool.tile([128, V], BF16, name=f"kt{l}{q}")
                   for q in range(2)] for l in range(L)]
            vals = [[rpool.tile([128, 16, AD + 1], BF16, name=f"vals{l}{h}")
                     for h in range(H)] for l in range(L)]

            # =============================================================
            # Phase helpers
            # =============================================================
            def kv_chain(l, h, pre):
                """One (layer, head, k-or-v) MLP chain over all V rows."""
                w18d, w28d = kvw[pre + "18"], kvw[pre + "28"]
                w3d, wc4d = kvw[pre + "3"], kvw[pre + "c4"]
                cn = f"{pre}{l}{h}"
                w18 = wpool.tile([128, 2, M], FP8, tag="w18", name=f"w18{cn}")
                w28 = wpool.tile([128, 2, M], FP8, tag="w28", name=f"w28{cn}")
                w3a = wpool.tile([128, AD], BF16, tag="w3a", name=f"w3a{cn}")
                w3b = wpool.tile([128, AD], BF16, tag="w3b", name=f"w3b{cn}")
                wc4 = wpool.tile([128, M], BF16, tag="wc4", name=f"wc4{cn}")
                nc.sync.dma_start(
                    w18[:], w18d[l, h].rearrange("i p m -> p i m"))
                nc.sync.dma_start(
                    w28[:], w28d[l, h].rearrange("i p m -> p i m"))
                nc.sync.dma_start(w3a[:], w3d[l, h, 0:128, :])
                nc.sync.dma_start(w3b[:], w3d[l, h, 128:256, :])
                for k4 in range(4):
                    nc.sync.dma_start(wc4[32 * k4:32 * k4 + 1, :],
                                      wc4d[l, h, k4:k4 + 1, :])
                b1d = kb1d if pre == "k" else vb1d
                b2d = kb2d if pre == "k" else vb2d
                b1 = [wpool.tile([128, 1], mybir.dt.float32, tag=f"b1_{fc}",
                                 name=f"b1{cn}_{fc}") for fc in range(2)]
                b2 = [wpool.tile([128, 1], mybir.dt.float32, tag=f"b2_{fc}",
                                 name=f"b2{cn}_{fc}") for fc in range(2)]
                for fc in range(2):
                    nc.sync.dma_start(
                        b1[fc][:], b1d[l, h, fc, :].rearrange("(p o) -> p o", o=1))
                    nc.sync.dma_start(
                        b2[fc][:], b2d[l, h, fc, :].rearrange("(p o) -> p o", o=1))

                # mm1 -> h1 (relu, fp8 DR layout), per (fc, ntp)
                h18 = [kpool.tile([128, 2, 1024], FP8, tag=f"h18_{ntp}",
                                  name=f"h18{cn}{ntp}", bufs=3)
                       for ntp in range(2)]
                for fc in range(2):
                    ps1s = [pp.tile([128, 1024], mybir.dt.float32,
                                    tag="kv", name=f"ps1{cn}{fc}{ntp}",
                                    bufs=2) for ntp in range(2)]
                    # 4 rank-1 true_u rows on 4 distinct 32-row PE bands
                    # (concurrent), then 4 DR matmuls sharing one weight load
                    for ntp in range(2):
                        for j in range(2):
                            nt = ntp * 2 + j
                            nc.tensor.matmul(
                                ps1s[ntp][:, 512 * j:512 * (j + 1)],
                                wc4[32 * nt:32 * nt + 1,
                                    128 * fc:128 * (fc + 1)],
                                u4[32 * nt:32 * nt + 1, :],
                                start=True, stop=False,
                                tile_position=(32 * nt, 0))
                    for ntp in range(2):
                        for j in range(2):
                            nt = ntp * 2 + j
                            nc.tensor.matmul(
                                ps1s[ntp][:, 512 * j:512 * (j + 1)],
                                w18[:, :, 128 * fc:128 * (fc + 1)],
                                xt8[:, :, 512 * nt:512 * (nt + 1)],
                                start=False, stop=True, perf_mode=DR)
                    for ntp in range(2):
                        nc.scalar.activation(h18[ntp][:, fc, :], ps1s[ntp][:],
                                             AF.Relu, bias=b1[fc][:])

                # mm2 -> h2 (relu, bf16)
                h2t = {}
                for fc in range(2):
                    for ntp in range(2):
                        ps2 = pp.tile([128, 1024], mybir.dt.float32,
                                      tag="kv", name=f"ps2{cn}{fc}{ntp}",
                                      bufs=2)
                        for j in range(2):
                            nc.tensor.matmul(
                                ps2[:, 512 * j:512 * (j + 1)],
                                w28[:, :, 128 * fc:128 * (fc + 1)],
                                h18[ntp][:, :, 512 * j:512 * (j + 1)],
                                start=True, stop=True, perf_mode=DR)
                        t = kpool.tile([128, 1024], BF16, tag="h2",
                                       name=f"h2{cn}{fc}{ntp}", bufs=5)
                        if pre == "v" and (fc + ntp) % 2 == 1:
                            nc.scalar.activation(t[:], ps2[:], AF.Relu,
                                                 bias=b2[fc][:])
                        else:
                            nc.vector.tensor_scalar(
                                t[:], ps2[:], b2[fc][:], 0.0, ALU.add, ALU.max)
                        h2t[(fc, ntp)] = t

                if pre == "k":
                    q, hp = h // 4, h % 4
                    for nt in range(4):
                        ntp, j = nt // 2, nt % 2
                        col = slice(512 * j, 512 * (j + 1))
                        psk = pp.tile([AD, 512], mybir.dt.float32,
                                      tag="p512", name=f"psk{cn}{nt}", bufs=2)
                        nc.tensor.matmul(psk[:], w3a[:], h2t[(0, ntp)][:, col],
                                         start=True, stop=False)
                        nc.tensor.matmul(psk[:], w3b[:], h2t[(1, ntp)][:, col],
                                         start=False, stop=True)
                        if l == 0:
                            nc.scalar.copy(
                                kt[l][q][32 * hp:32 * (hp + 1),
                                         512 * nt:512 * (nt + 1)], psk[:])
                        else:
                            nc.vector.tensor_copy(
                                kt[l][q][32 * hp:32 * (hp + 1),
                                         512 * nt:512 * (nt + 1)], psk[:])
                else:
                    psv = pp.tile([128, 512], mybir.dt.float32,
                                  tag="p512", name=f"psv{cn}", bufs=2)
                    for nt in range(4):
                        ntp, j = nt // 2, nt % 2
                        for sv in range(4):
                            svg = nt * 4 + sv
                            vsl = slice(512 * j + 128 * sv, 512 * j + 128 * (sv + 1))
                            osl = slice(32 * svg, 32 * (svg + 1))
                            first = (svg == 0)
                            last = (svg == 15)
                            nc.tensor.matmul(
                                psv[:, osl], h2t[(0, ntp)][:, vsl], w3a[:],
                                start=first, stop=False)
                            nc.tensor.matmul(
                                psv[:, osl], h2t[(1, ntp)][:, vsl], w3b[:],
                                start=False, stop=last)
                    vt = vals[l][h]
                    nc.vector.tensor_copy(
                        vt[:, :, 0:AD],
                        psv[:].rearrange("p (s d) -> p s d", d=AD))
                    nc.vector.memset(vt[:, :, AD:AD + 1], 1.0)

            # attv tiles (f32) + bf16/lo splits, rotated per layer
            def split_bf(src_tiles, tagp, need_lo=True):
                """f32 [128,P] tiles -> (hi bf16, lo bf16) tiles."""
                his, los = [], []
                for q, s in enumerate(src_tiles):
                    hi = kpool.tile([128, P], BF16, tag=f"{tagp}h{q}",
                                    name=f"{tagp}h{q}", bufs=2, uniquify=True)
                    nc.vector.tensor_copy(hi[:], s[:])
                    his.append(hi)
                    if need_lo:
                        lo = kpool.tile([128, P], BF16, tag=f"{tagp}l{q}",
                                        name=f"{tagp}l{q}", bufs=2, uniquify=True)
                        nc.vector.tensor_tensor(lo[:], s[:], hi[:], ALU.subtract)
                        los.append(lo)
                return his, los

            def layer_norm(xq, gd, bd, l, nm, cols=None, tail=False):
                """T-layout LN over 256 features; returns new f32 tiles.
                cols: list of column slices processed as independent
                pipelined chunks (tail latency hiding)."""
                if cols is None:
                    cols = [slice(0, P)]
                xh = [kpool.tile([128, P], BF16, tag=f"lnxh{q}",
                                 name=f"lnxh{nm}{q}", bufs=2, uniquify=True)
                      for q in range(2)]
                sq = [kpool.tile([128, P], BF16, tag=f"lnsq{q}",
                                 name=f"lnsq{nm}{q}", bufs=1) for q in range(2)]
                mu = kpool.tile([1, P], mybir.dt.float32, tag="lnmu", bufs=1,
                                name=f"lnmu{nm}")
                m2 = kpool.tile([1, P], mybir.dt.float32, tag="lnm2", bufs=1,
                                name=f"lnm2{nm}")
                var = kpool.tile([1, P], mybir.dt.float32, tag="lnvar", bufs=1,
                                 name=f"lnvar{nm}")
                sd = kpool.tile([1, P], mybir.dt.float32, tag="lnsd", bufs=1,
                                name=f"lnsd{nm}")
                rstd = kpool.tile([1, P], BF16, tag="lnrs", bufs=1,
                                  name=f"lnrs{nm}")
                nmu = kpool.tile([1, P], BF16, tag="lnnm", bufs=1,
                                 name=f"lnnm{nm}")
                t1 = [kpool.tile([128, P], mybir.dt.float32, tag=f"lnt{q}",
                                 name=f"lnt{nm}{q}", bufs=1) for q in range(2)]
                outq = [kpool.tile([128, P], mybir.dt.float32, tag=f"attv{q}",
                                   name=f"ln_out{nm}{q}", bufs=2)
                        for q in range(2)]
                gt, bt = [], []
                for q in range(2):
                    g = wpool.tile([128, 1], mybir.dt.float32, tag=f"lng{q}",
                                   name=f"lng{nm}{q}")
                    nc.sync.dma_start(g[:], gd[l, q, :, :])
                    bb = wpool.tile([128, 1], mybir.dt.float32, tag=f"lnb{q}",
                                    name=f"lnb{nm}{q}")
                    nc.sync.dma_start(bb[:], bd[l, q, :, :])
                    gt.append(g); bt.append(bb)
                for cs in cols:
                    for q in range(2):
                        if tail:
                            nc.scalar.copy(xh[q][:, cs], xq[q][:, cs])
                        else:
                            nc.vector.tensor_copy(xh[q][:, cs], xq[q][:, cs])
                    pst = pp.tile([1, 512], mybir.dt.float32, tag="p512",
                                  name=f"lnsum{nm}{cs.start}", bufs=2)
                    w = cs.stop - cs.start
                    nc.tensor.matmul(pst[:, 0:w], ones_c128b[:], xh[0][:, cs],
                                     start=True, stop=False)
                    nc.tensor.matmul(pst[:, 0:w], ones_c128b[:], xh[1][:, cs],
                                     start=False, stop=True)
                    for q in range(2):
                        nc.vector.tensor_tensor(sq[q][:, cs], xh[q][:, cs],
                                                xh[q][:, cs], ALU.mult)
                    psq = pp.tile([1, 512], mybir.dt.float32, tag="p512",
                                  name=f"lnsq{nm}{cs.start}", bufs=2)
                    nc.tensor.matmul(psq[:, 0:w], ones_c128b[:], sq[0][:, cs],
                                     start=True, stop=False)
                    nc.tensor.matmul(psq[:, 0:w], ones_c128b[:], sq[1][:, cs],
                                     start=False, stop=True)
                    nc.scalar.mul(mu[:, cs], pst[:, 0:w], 1.0 / D)
                    nc.vector.tensor_tensor(m2[:, cs], mu[:, cs], mu[:, cs],
                                            ALU.mult)
                    nc.vector.scalar_tensor_tensor(
                        var[:, cs], psq[:, 0:w], 1.0 / D, m2[:, cs],
                        ALU.mult, ALU.subtract)
                    nc.scalar.activation(sd[:, cs], var[:, cs], AF.Sqrt,
                                         bias=eps_t[:])
                    with nc.allow_low_precision(reason="bf16 LN bcast"):
                        nc.vector.reciprocal(rstd[:, cs], sd[:, cs])
                    nc.vector.scalar_tensor_tensor(
                        nmu[:, cs], mu[:, cs], -1.0, rstd[:, cs],
                        ALU.mult, ALU.mult)
                    psa = pp.tile([128, 512], mybir.dt.float32, tag="p512",
                                  name=f"lnA{nm}{cs.start}", bufs=2)
                    nc.tensor.matmul(psa[:, 0:w], ones_r128[:], rstd[:, cs],
                                     start=True, stop=True)
                    psb = pp.tile([128, 512], mybir.dt.float32, tag="p512",
                                  name=f"lnB{nm}{cs.start}", bufs=2)
                    nc.tensor.matmul(psb[:, 0:w], ones_r128[:], nmu[:, cs],
                                     start=True, stop=True)
                    for q in range(2):
                        nc.vector.tensor_tensor(t1[q][:, cs], xq[q][:, cs],
                                                psa[:, 0:w], ALU.mult)
                        nc.vector.tensor_tensor(t1[q][:, cs], t1[q][:, cs],
                                                psb[:, 0:w], ALU.add)
                        if tail:
                            nc.scalar.activation(outq[q][:, cs], t1[q][:, cs],
                                                 AF.Identity, scale=gt[q][:],
                                                 bias=bt[q][:])
                        else:
                            nc.vector.tensor_scalar(outq[q][:, cs],
                                                    t1[q][:, cs],
                                                    gt[q][:], bt[q][:],
                                                    ALU.mult, ALU.add)
                return outq

            def attention(l, attv):
                """One attention block (scores/softmax/values) -> xres."""
                qt_h, _ = split_bf(attv, "qt", need_lo=False)
                numer = [kpool.tile([128, P], mybir.dt.float32, tag=f"num{q}",
                                    name=f"numer{l}{q}", bufs=1)
                         for q in range(2)]
                dn8 = kpool.tile([8, P], mybir.dt.float32, tag="dn8",
                                 name=f"dn8{l}", bufs=1)
                for pair in range(4):
                    h0 = 2 * pair
                    h1 = h0 + 1
                    q = h0 // 4
                    b0, b1r = 32 * (h0 % 4), 32 * (h1 % 4)
                    psA = pp.tile([128, 1024], mybir.dt.float32, tag="acc",
                                  name=f"psA{l}{pair}", bufs=1)
                    for vc in range(16):
                        pss = pp.tile([128, 1024], mybir.dt.float32,
                                      tag="kv", name=f"pss{l}{pair}{vc}",
                                      bufs=2)
                        for hi, (hh, bb) in enumerate(((h0, b0), (h1, b1r))):
                            tp = (bb, 0) if bb == 96 else None
                            nc.tensor.matmul(
                                pss[:, 512 * hi:512 * (hi + 1)],
                                kt[l][q][bb:bb + 32, 128 * vc:128 * (vc + 1)],
                                qt_h[q][bb:bb + 32, :],
                                start=True, stop=True, tile_position=tp)
                        e = kpool.tile([128, 1024], BF16, tag="ebuf",
                                       name=f"e{l}{pair}{vc}", bufs=2)
                        nc.scalar.activation(e[:], pss[:], AF.Exp,
                                             scale=SCALE)
                        ce = kpool.tile([128, 1024], BF16, tag="cebuf",
                                        name=f"ce{l}{pair}{vc}", bufs=2)
                        nc.vector.tensor_tensor(
                            ce[:], e[:],
                            ct2[:, vc, :, :].rearrange("p a q -> p (a q)"),
                            ALU.mult)
                        nc.tensor.matmul(
                            psA[0:AD + 1, 0:512],
                            vals[l][h0][:, vc, :], ce[:, 0:512],
                            start=(vc == 0), stop=(vc == 15))
                        nc.tensor.matmul(
                            psA[64:64 + AD + 1, 512:1024],
                            vals[l][h1][:, vc, :], ce[:, 512:1024],
                            start=(vc == 0), stop=(vc == 15),
                            tile_position=(0, 64))
                    nc.vector.tensor_copy(numer[q][b0:b0 + 32, :],
                                          psA[0:32, 0:512])
                    nc.vector.tensor_copy(numer[q][b1r:b1r + 32, :],
                                          psA[64:96, 512:1024])
                    for hh, prow, csl in ((h0, 32, slice(0, 512)),
                                          (h1, 96, slice(512, 1024))):
                        dtmp = kpool.tile([1, P], mybir.dt.float32,
                                          tag="dntmp", bufs=2, name=f"dtmp{l}{hh}",
                                          uniquify=True)
                        nc.vector.tensor_copy(dtmp[:], psA[prow:prow + 1, csl])
                        nc.sync.dma_start(dn8[hh:hh + 1, :], dtmp[:])

                rd8 = kpool.tile([8, P], BF16, tag="rd8",
                                 name=f"rd8{l}", bufs=1)
                with nc.allow_low_precision(reason="bf16 softmax recip bcast"):
                    nc.vector.reciprocal(rd8[:], dn8[:])
                vb3q = [wpool.tile([128, 1], mybir.dt.float32, tag=f"vb3{q}",
                                   name=f"vb3{l}{q}") for q in range(2)]
                for q in range(2):
                    nc.sync.dma_start(vb3q[q][:], vb3c[l, q, :, :])
                xres = []
                for q in range(2):
                    psrb = pp.tile([128, 512], mybir.dt.float32, tag="p512",
                                   name=f"psrb{l}{q}", bufs=2)
                    nc.tensor.matmul(psrb[:], oh8[:, 128 * q:128 * (q + 1)],
                                     rd8[:], start=True, stop=True)
                    t1 = kpool.tile([128, P], mybir.dt.float32, tag=f"xres{q}",
                                    name=f"xres{l}{q}", bufs=1)
                    nc.vector.tensor_tensor(t1[:], numer[q][:], psrb[:], ALU.mult)
                    nc.vector.tensor_scalar(t1[:], t1[:], vb3q[q][:], None,
                                            ALU.add)
                    nc.vector.tensor_tensor(t1[:], attv[q][:], t1[:], ALU.add)
                    xres.append(t1)
                return xres

            def ff_block(l, attv, cols=None, tail=False):
                if cols is None:
                    cols = [slice(0, P)]
                fw1h = [wpool.tile([128, D], BF16, tag=f"fw1h{kc}",
                                   name=f"fw1h{l}{kc}") for kc in range(2)]
                fw2h = [wpool.tile([128, D], BF16, tag=f"fw2h{kc}",
                                   name=f"fw2h{l}{kc}") for kc in range(2)]
                for kc in range(2):
                    sl = slice(128 * kc, 128 * (kc + 1))
                    nc.sync.dma_start(fw1h[kc][:], ffw1h[l, sl, :])
                    nc.sync.dma_start(fw2h[kc][:], ffw2h[l, sl, :])
                fb1 = [wpool.tile([128, 1], F32, tag=f"fb1_{fc}",
                                  name=f"fb1{l}{fc}") for fc in range(2)]
                fb2 = [wpool.tile([128, 1], F32, tag=f"fb2_{fc}",
                                  name=f"fb2{l}{fc}") for fc in range(2)]
                for fc in range(2):
                    nc.sync.dma_start(fb1[fc][:], ffb1c[l, fc, :, :])
                    nc.sync.dma_start(fb2[fc][:], ffb2c[l, fc, :, :])

                av_h = [kpool.tile([128, P], BF16, tag=f"ffsh{q}",
                                   name=f"ffsh{l}{q}", bufs=2, uniquify=True)
                        for q in range(2)]
                hh_t = [kpool.tile([128, P], BF16, tag=f"ffhh{fc}",
                                   name=f"ffhh{l}{fc}", bufs=2)
                        for fc in range(2)]
                xres2 = [kpool.tile([128, P], mybir.dt.float32, tag=f"xres{fc}",
                                    name=f"xr2{l}{fc}", bufs=1)
                         for fc in range(2)]
                t2a = [kpool.tile([128, P], mybir.dt.float32, tag=f"t2a{fc}",
                                  name=f"t2a{l}{fc}", bufs=1)
                       for fc in range(2)]
                for cs in cols:
                    w = cs.stop - cs.start
                    for q in range(2):
                        if tail:
                            nc.scalar.copy(av_h[q][:, cs], attv[q][:, cs])
                        else:
                            nc.vector.tensor_copy(av_h[q][:, cs], attv[q][:, cs])
                    for fc in range(2):
                        psf = pp.tile([128, 512], mybir.dt.float32, tag="p512",
                                      name=f"psff1{l}{fc}{cs.start}", bufs=2)
                        for kc in range(2):
                            nc.tensor.matmul(
                                psf[:, 0:w], fw1h[kc][:, 128 * fc:128 * (fc + 1)],
                                av_h[kc][:, cs], start=(kc == 0), stop=(kc == 1))
                        nc.scalar.activation(hh_t[fc][:, cs], psf[:, 0:w],
                                             AF.Relu, bias=fb1[fc][:])
                    for fc in range(2):
                        psf2 = pp.tile([128, 512], mybir.dt.float32, tag="p512",
                                       name=f"psff2{l}{fc}{cs.start}", bufs=2)
                        for kc in range(2):
                            nc.tensor.matmul(
                                psf2[:, 0:w], fw2h[kc][:, 128 * fc:128 * (fc + 1)],
                                hh_t[kc][:, cs], start=(kc == 0), stop=(kc == 1))
                        nc.scalar.activation(t2a[fc][:, cs], psf2[:, 0:w],
                                             AF.Identity, bias=fb2[fc][:])
                        nc.vector.tensor_tensor(xres2[fc][:, cs],
                                                attv[fc][:, cs],
                                                t2a[fc][:, cs], ALU.add)
                return xres2

            # =============================================================
            # Emit program
            # =============================================================
            # ds projection: attv0 = cur @ ds_W + ds_b   (T-layout out)
            dsw_h = [cpool.tile([128, D], BF16, name=f"dswh{kc}") for kc in range(2)]
            for kc in range(2):
                nc.sync.dma_start(dsw_h[kc][:], dswh[128 * kc:128 * (kc + 1), :])
            dsb_q = [cpool.tile([128, 1], F32, name=f"dsb{q}") for q in range(2)]
            for q in range(2):
                nc.sync.dma_start(dsb_q[q][:], dsbd[q, :, :])

            attv = []
            for q in range(2):
                psd = pp.tile([128, 512], mybir.dt.float32, tag="p512",
                              name=f"psds{q}", bufs=2)
                for kc in range(2):
                    nc.tensor.matmul(psd[:], dsw_h[kc][:, 128 * q:128 * (q + 1)],
                                     cur_h[kc][:], start=(kc == 0),
                                     stop=(kc == 1))
                o = kpool.tile([128, P], mybir.dt.float32, tag=f"attv{q}",
                               name=f"attv0{q}", bufs=2)
                nc.scalar.activation(o[:], psd[:], AF.Identity,
                                     bias=dsb_q[q][:])
                attv.append(o)

            # l0 KV, then l1 keys
            for q in range(2):
                for hp in range(4):
                    kv_chain(0, 4 * q + hp, "k")
                for hp in range(4):
                    kv_chain(0, 4 * q + hp, "v")
            for cc in range(2):
                nc.sync.dma_start(
                    ct2[:, :, cc, :], ctm.rearrange("(c p) q -> p c q", p=128))
            for q in range(2):
                for hp in range(4):
                    kv_chain(1, 4 * q + hp, "k")

            # layer 0 attention + FF (overlaps l1 v-chains below)
            xres = attention(0, attv)
            attv = layer_norm(xres, ln1gd, ln1bd, 0, "ln1_0")

            # l1 v-chains: emitted after l0 attention so the scheduler
            # backfills PE bubbles during the scalar-bound attention window;
            # the q1 half goes after LN2 to fill the pre-l1-attention dip
            for hp in range(4):
                kv_chain(1, hp, "v")

            xres2 = ff_block(0, attv)
            attv = layer_norm(xres2, ln2gd, ln2bd, 0, "ln2_0")
            for hp in range(4):
                kv_chain(1, 4 + hp, "v")

            # layer 1
            HALVES = [slice(0, 256), slice(256, 512)]
            xres = attention(1, attv)
            attv = layer_norm(xres, ln1gd, ln1bd, 1, "ln1_1", cols=HALVES,
                              tail=True)
            xres2 = ff_block(1, attv, cols=HALVES, tail=True)
            attv = layer_norm(xres2, ln2gd, ln2bd, 1, "ln2_1", cols=HALVES,
                              tail=True)

            # ---- decoder --------------------------------------------------
            dw1h = [cpool.tile([128, M], BF16, name=f"dw1h{kc}") for kc in range(2)]
            dw2h = [cpool.tile([128, M], BF16, name=f"dw2h{kc}") for kc in range(2)]
            dw3h = [cpool.tile([128, R], BF16, name=f"dw3h{kc}") for kc in range(2)]
            for kc in range(2):
                sl = slice(128 * kc, 128 * (kc + 1))
                nc.sync.dma_start(dw1h[kc][:], dew1h[sl, :])
                nc.sync.dma_start(dw2h[kc][:], dew2h[sl, :])
                nc.sync.dma_start(dw3h[kc][:], dew3h[sl, :])
            db1 = [cpool.tile([128, 1], F32, name=f"db1{fc}") for fc in range(2)]
            db2 = [cpool.tile([128, 1], F32, name=f"db2{fc}") for fc in range(2)]
            for fc in range(2):
                nc.sync.dma_start(db1[fc][:], deb1c[fc, :, :])
                nc.sync.dma_start(db2[fc][:], deb2c[fc, :, :])
            db3h = cpool.tile([1, R], BF16, name="db3h")
            nc.sync.dma_start(db3h[:], deb3h[:])

            de_h = [kpool.tile([128, P], BF16, tag=f"ffsh{q}",
                                name=f"desh{q}", bufs=2, uniquify=True)
                    for q in range(2)]
            d1h = [kpool.tile([128, P], BF16, tag=f"d1h{fc}",
                              name=f"d1h{fc}", bufs=1) for fc in range(2)]
            d2h = [kpool.tile([128, P], BF16, tag=f"d2h{fc}",
                              name=f"d2h{fc}", bufs=1) for fc in range(2)]
            for cs in HALVES:
                w = cs.stop - cs.start
                for q in range(2):
                    nc.scalar.copy(de_h[q][:, cs], attv[q][:, cs])
                for fc in range(2):
                    psd1 = pp.tile([128, 512], mybir.dt.float32, tag="p512",
                                   name=f"psde1{fc}{cs.start}", bufs=2)
                    for kc in range(2):
                        nc.tensor.matmul(
                            psd1[:, 0:w], dw1h[kc][:, 128 * fc:128 * (fc + 1)],
                            de_h[kc][:, cs], start=(kc == 0), stop=(kc == 1))
                    nc.scalar.activation(d1h[fc][:, cs], psd1[:, 0:w],
                                         AF.Relu, bias=db1[fc][:])
                for fc in range(2):
                    psd2 = pp.tile([128, 512], mybir.dt.float32, tag="p512",
                                   name=f"psde2{fc}{cs.start}", bufs=2)
                    for kc in range(2):
                        nc.tensor.matmul(
                            psd2[:, 0:w], dw2h[kc][:, 128 * fc:128 * (fc + 1)],
                            d1h[kc][:, cs], start=(kc == 0), stop=(kc == 1))
                    nc.scalar.activation(d2h[fc][:, cs], psd2[:, 0:w],
                                         AF.Relu, bias=db2[fc][:])

            # logits row-major [p, R] per 128-p chunk + loss
            t4 = kpool.tile([128, 4], mybir.dt.float32, tag="t4",
                            name="t4", bufs=1)
            for pc in range(4):
                psl = pp.tile([128, R], mybir.dt.float32, tag="p512",
                              name=f"pslog{pc}", bufs=2)
                nc.tensor.matmul(psl[:], ones_r128[:], db3h[:],
                                 start=True, stop=False)
                psl_sl = slice(128 * pc, 128 * (pc + 1))
                for kc in range(2):
                    nc.tensor.matmul(psl[:], d2h[kc][:, psl_sl], dw3h[kc][:],
                                     start=False, stop=(kc == 1))
                escr = kpool.tile([128, R], mybir.dt.float32, tag="escr", bufs=1,
                                  name=f"escr{pc}")
                se = kpool.tile([128, 1], mybir.dt.float32, tag="se",
                                name=f"se{pc}", bufs=2)
                nc.scalar.activation(escr[:], psl[:], AF.Exp,
                                     accum_out=se[:])
                ls = kpool.tile([128, 1], mybir.dt.float32, tag="ls",
                                name=f"ls{pc}", bufs=2)
                nc.scalar.activation(ls[:], se[:], AF.Ln)
                oht = kpool.tile([128, R], BF16, tag="ohh", bufs=2,
                                 name=f"oht{pc}")
                nc.sync.dma_start(oht[:], ohtd[psl_sl, :])
                scr2 = kpool.tile([128, R], mybir.dt.float32, tag="scr2", bufs=1,
                                  name=f"scr2{pc}")
                pk = kpool.tile([128, 1], mybir.dt.float32, tag="pk",
                                name=f"pk{pc}", bufs=2)
                nc.vector.scalar_tensor_tensor(
                    scr2[:], psl[:], 1.0, oht[:], ALU.mult, ALU.mult,
                    accum_out=pk[:])
                nc.vector.tensor_tensor(t4[:, pc:pc + 1], pk[:], ls[:],
                                        ALU.subtract)
            pspr = pp.tile([1, 4], mybir.dt.float32, tag="p512", name="pspr", bufs=2)
            nc.tensor.matmul(pspr[:], ones_c128f[:], t4[:], start=True, stop=True)
            pr4 = kpool.tile([1, 4], mybir.dt.float32, tag="pr4",
                             name="pr4", bufs=1)
            nc.vector.tensor_copy(pr4[:], pspr[:])
            s1 = kpool.tile([1, 1], mybir.dt.float32, tag="s1",
                            name="s1", bufs=1)
            nc.vector.tensor_reduce(s1[:], pr4[:], mybir.AxisListType.X, ALU.add)
            outt = kpool.tile([1, 1], mybir.dt.float32, tag="outt",
                              name="outt", bufs=1)
            nc.scalar.activation(outt[:], s1[:], AF.Identity,
                                 bias=nlogr_t[:], scale=-1.0)
            nc.sync.dma_start(out_d[:], outt[:])

    return nc


def _split(x):
    h = np.asarray(x, np.float32).astype(BF)
    lo = (np.asarray(x, np.float32) - h.astype(np.float32)).astype(BF)
    return h, lo


def _maybe_enable_trace():
    """Optional NTFF profiling under axon (KERNEL_TRACE=1); best-effort."""
    try:
        import sys
        import types

        import antenv

        if "antenv.axon_hooks" not in sys.modules:
            mod = types.ModuleType("antenv.axon_hooks")
            mod._hook = None
            mod.set_axon_ntff_profile_hook = lambda h: setattr(mod, "_hook", h)
            mod.get_axon_ntff_profile_hook = lambda: mod._hook
            sys.modules["antenv.axon_hooks"] = mod
            antenv.axon_hooks = mod
            from trn_agent_boot.trn_boot import _ntff_profile_via_ctypes

            mod._hook = _ntff_profile_via_ctypes("/opt/axon/libaxon_pjrt.so")
        import concourse.bass_utils as _bu

        _bu.upload_artifacts = lambda tmpdir: f"file://{tmpdir}"
        return True
    except Exception:
        return False


LAST_RESULT = {}


def kernel(**inputs):
    from concourse.bass_utils import run_bass_kernel_spmd

    if "nc" not in _BUILT:
        _BUILT["nc"] = _build()
    nc = _BUILT["nc"]

    f32 = lambda a: np.ascontiguousarray(np.asarray(a, np.float32))
    bf = lambda a: np.ascontiguousarray(np.asarray(a, np.float32)).astype(BF)
    f8 = lambda a: np.ascontiguousarray(np.asarray(a, np.float32)).astype(F8)

    enc = f32(inputs["encoded"])                      # [B,V,I]
    tu = f32(inputs["true_u"])                        # [B,V,1]
    mask = f32(inputs["attn_mask"])                   # [P,N]
    pp_ = np.asarray(inputs["pred_points"]).astype(np.int64)
    ni = np.asarray(inputs["neighbor_index"]).astype(np.int64)

    # count matrix C[p, v]
    C = np.zeros((P, V), np.float32)
    np.add.at(C, (np.repeat(np.arange(P), N), ni.ravel()),
              np.exp(-SCALE * mask).ravel().astype(np.float32))
    ctm = np.ascontiguousarray(C.T).astype(BF)        # [V, P]

    shared = {"ctm": ctm}
    for pre in ("k", "v"):
        W1 = f32(inputs[pre + "W1"])                  # [L,H,257,M]
        W2 = f32(inputs[pre + "W2"])                  # [L,H,M,M]
        shared[pre + "w18"] = f8(
            W1[:, :, :256, :].reshape(L, H, 2, 128, M))
        shared[pre + "w28"] = f8(W2.reshape(L, H, 2, 128, M))
        shared[pre + "w3"] = bf(inputs[pre + "W3"])
        shared[pre + "c4"] = bf(
            np.broadcast_to(W1[:, :, 256:257, :], (L, H, 4, M)))
    shared["kb1d"] = f32(inputs["kb1"]).reshape(L, H, 2, 128)
    shared["kb2d"] = f32(inputs["kb2"]).reshape(L, H, 2, 128)
    shared["vb1d"] = f32(inputs["vb1"]).reshape(L, H, 2, 128)
    shared["vb2d"] = f32(inputs["vb2"]).reshape(L, H, 2, 128)
    shared["vb3c"] = np.ascontiguousarray(
        f32(inputs["vb3"]).reshape(L, 2, 128, 1))     # [l, q, hp*32+d, 1]

    for nm, key in (("dsw", "ds_W"), ("dew1", "de_W1"), ("dew2", "de_W2"),
                    ("dew3", "de_W3"), ("ffw1", "ff_W1"), ("ffw2", "ff_W2")):
        shared[nm + "h"] = bf(inputs[key])
    shared["dsbd"] = f32(inputs["ds_b"]).reshape(2, 128, 1)
    shared["ffb1c"] = f32(inputs["ff_b1"]).reshape(L, 2, 128, 1)
    shared["ffb2c"] = f32(inputs["ff_b2"]).reshape(L, 2, 128, 1)
    shared["deb1c"] = f32(inputs["de_b1"]).reshape(2, 128, 1)
    shared["deb2c"] = f32(inputs["de_b2"]).reshape(2, 128, 1)
    shared["deb3h"] = bf(inputs["de_b3"]).reshape(1, R)
    shared["ln1gd"] = f32(inputs["ln1_g"]).reshape(L, 2, 128, 1)
    shared["ln1bd"] = f32(inputs["ln1_b"]).reshape(L, 2, 128, 1)
    shared["ln2gd"] = f32(inputs["ln2_g"]).reshape(L, 2, 128, 1)
    shared["ln2bd"] = f32(inputs["ln2_b"]).reshape(L, 2, 128, 1)

    oh8f = np.zeros((8, D), np.float32)
    for hh in range(8):
        base = 128 * (hh // 4) + 32 * (hh % 4)
        oh8f[hh, base:base + 32] = 1.0
    shared["oh8d"] = oh8f.astype(BF)

    in_maps = []
    for b in range(B):
        merged = np.concatenate([enc[b], tu[b]], axis=1)  # [V, 257]
        mt = np.ascontiguousarray(merged.T)               # [257, V]
        cur = enc[b][pp_, :]                              # [P, I]
        curt = np.ascontiguousarray(cur.T)                # [I, P]
        m = dict(shared)
        m["xt8d"] = np.ascontiguousarray(
            mt[0:256].reshape(2, 128, V).transpose(1, 0, 2)).astype(F8)
        m["u4d"] = np.ascontiguousarray(
            mt[256].reshape(4, 512)).astype(BF)
        m["curh"] = curt.astype(BF)
        tgt = np.clip(np.floor(tu[b][pp_, 0] * R).astype(np.int64), 0, R - 1)
        ohp = np.zeros((P, R), np.float32)
        ohp[np.arange(P), tgt] = 1.0
        m["ohtd"] = ohp.astype(BF)
        in_maps.append(m)

    trace = os.environ.get("KERNEL_TRACE") == "1" and _maybe_enable_trace()
    res = run_bass_kernel_spmd(
        nc, in_maps, core_ids=list(range(B)), trace=trace,
        trace_cores=list(range(B)) if trace else None)
    LAST_RESULT["res"] = res
    if trace and res.exec_time_ns is not None:
        print(f"HW exec time: {res.exec_time_ns} ns "
              f"(mean {res.mean_exec_time_ns} ns, "
              f"slowest core {res.max_exec_time_core_id})")
    out = np.array([res.results[b]["out"][0, 0] for b in range(B)], np.float32)
    return out
